# revision 9
# baseline (speedup 1.0000x reference)
"""AdaptiveGraphAttention TRN2 kernel: 8-core data parallel (b = core//2, row-half = core%2).

Math notes (factorizations used, all exact up to f32 rounding):
  - edge MLP layer1 on concat([desc_i, desc_j]) splits: e1 = P_i + Q_j + b1,
    P = desc @ W1a, Q = desc @ W1b (+b1).
  - LayerNorm stats: mu_ij = mP_i + mQ_j; var_ij = varP_i + varQ_j + (2/C)*(Pc_i . Qc_j),
    so var comes from one [S,S] matmul C = Pc @ Qc^T.
  - beta==0 => relu(((e1-mu)/sig)*g) = (1/sig)*relu(g*(e1-mu)); sig>0.
  - layer2 + per-head dot with we collapses: (relu_e @ eu_w2 + b2) . we_h
      = relu_e @ W2E[:,h] + b2we_h, W2E = eu_w2 @ blockdiag(we) -> [256, 8].
  - qterm (q . wq per (i,h)) is constant along j => softmax-invariant => dropped.
    Same for ap_b. Both verified invariant (also masked rows swallow them in f32).
Each core runs the identical program on 65 row-slots; slot0 is CLS (half 0,
blended via data flags) or a duplicated row (half 1, dropped by host).
"""
import numpy as np

B, S, D, H, HID = 4, 128, 256, 8, 256
NS = S + 1
HD = D // H
NEG = -1e9
NSLOT = 65
F32 = np.float32


# ---------------------------------------------------------------------------
# host-side input prep
# ---------------------------------------------------------------------------
def _prep_core_inputs(inputs):
    """Returns list of 8 dicts (per-core input maps), all float32."""
    g = lambda k: np.asarray(inputs[k], F32)
    desc = g("desc_embeddings")           # [B,S,D]
    nv = g("name_value_embeddings")       # [B,NS,D]
    eu_w1 = g("eu_w1")                    # [2D, D]
    W1a, W1b = eu_w1[:D], eu_w1[D:]
    ap_w = g("ap_w")
    wq, wk, we = ap_w[:HD], ap_w[HD:2 * HD], ap_w[2 * HD:]
    eu_beta = g("eu_beta")
    assert np.abs(eu_beta).max() == 0.0, "kernel assumes eu_beta == 0"
    # folds
    W2E = np.stack([g("eu_w2")[:, h * HD:(h + 1) * HD] @ we for h in range(H)], 1)  # [D,8]
    Wk8 = np.stack([g("k_w")[:, h * HD:(h + 1) * HD] @ wk for h in range(H)], 1)    # [D,8]
    kbwk = np.array([g("k_b")[h * HD:(h + 1) * HD] @ wk for h in range(H)], F32)    # [8]
    b2we = np.array([g("eu_b2")[h * HD:(h + 1) * HD] @ we for h in range(H)], F32)  # [8]
    alpha = float(np.clip(np.asarray(inputs["alpha_param"], F32), 1e-5, 1.0))
    tb = float(np.asarray(inputs["topology_bias"], F32).reshape(-1)[0])

    def ksplit(w, n):  # [D, n] -> [2,128,n]
        return np.ascontiguousarray(w.reshape(2, 128, n))

    def col2(b):  # [256] -> [2,128,1]
        return np.ascontiguousarray(b.reshape(2, 128, 1))

    b2we128 = np.tile(b2we, 16).reshape(128, 1).astype(F32)       # partition 8r+h -> b2we[h]
    sel16 = np.zeros((16, 128), F32)
    for r in range(16):
        sel16[r, 8 * r:8 * r + 8] = 1.0
    sel8 = np.zeros((8, 128), F32)
    for r in range(16):
        for h in range(H):
            sel8[h, 8 * r + h] = 1.0
    shared = dict(
        th_w1=ksplit(g("th_w1"), HID), th_w2=ksplit(g("th_w2"), HID),
        tt_w1=ksplit(g("tt_w1"), HID), tt_w2=ksplit(g("tt_w2"), HID),
        ah_w=ksplit(g("ah_w"), HID), at_w=ksplit(g("at_w"), HID),
        w1a=ksplit(W1a, D), w1b=ksplit(W1b, D),
        v_w=ksplit(g("v_w"), D), out_w=ksplit(g("out_w"), D),
        wk8=ksplit(Wk8, 8), w2e=ksplit(W2E, 8),
        th_b1c=col2(g("th_b1")), th_b2c=col2(g("th_b2")),
        tt_b1c=col2(g("tt_b1")), tt_b2c=col2(g("tt_b2")),
        ah_bc=col2(g("ah_b")), at_bc=col2(g("at_b")),
        eu_b1c=col2(g("eu_b1")), g_c=col2(g("eu_g")),
        kbwk=kbwk.reshape(8, 1), b2we128=b2we128,
        vbias_rep=np.tile(g("v_b"), (128, 1)).astype(F32),
        obias_rep=np.tile(g("out_b"), (16, 1)).astype(F32),
        tb_col=np.full((NSLOT, 1), tb, F32),
        alpha_col=np.full((NSLOT, 1), alpha, F32),
        eps_col=np.full((NSLOT, 1), 1e-5, F32),
        ones_col=np.ones((128, 1), F32),
        ones_scl=np.full((128, 1), 1.0 / 256.0, F32),
        ones_row=np.ones((1, 128), F32),
        one1=np.ones((1, 1), F32),
        sel16=sel16, sel8=sel8,
        ident=np.eye(128, dtype=F32),
    )
    maps = []
    for core in range(8):
        b, half = core // 2, core % 2
        gsel = ([0] + list(range(64))) if half == 0 else ([63] + list(range(64, 128)))
        gsel = np.array(gsel)
        E = np.zeros((NSLOT, S), F32)
        E[np.arange(NSLOT), gsel] = 1.0
        m = dict(shared)
        m.update(
            desc_T=np.ascontiguousarray(desc[b].T),                    # [256,128]
            desc_T_loc=np.ascontiguousarray(desc[b][gsel].T),          # [256,65]
            nv_T=np.ascontiguousarray(nv[b].T),                        # [256,129]
            nv_T_loc=np.ascontiguousarray(nv[b][1 + gsel].T),          # [256,65]
            Einv=(1.0 - E), Eneg=(NEG * E).astype(F32),
            flag8=np.full((8, 1), 1.0 - half, F32),
            flaginv8=np.full((8, 1), float(half), F32),
            flag1=np.full((1, 1), 1.0 - half, F32),
            flaginv1=np.full((1, 1), float(half), F32),
        )
        maps.append(m)
    return maps


INPUT_SPECS = {
    "desc_T": (256, 128), "desc_T_loc": (256, NSLOT), "nv_T": (256, NS),
    "nv_T_loc": (256, NSLOT),
    "th_w1": (2, 128, HID), "th_w2": (2, 128, HID), "tt_w1": (2, 128, HID),
    "tt_w2": (2, 128, HID), "ah_w": (2, 128, HID), "at_w": (2, 128, HID),
    "w1a": (2, 128, D), "w1b": (2, 128, D), "v_w": (2, 128, D),
    "out_w": (2, 128, D), "wk8": (2, 128, 8), "w2e": (2, 128, 8),
    "th_b1c": (2, 128, 1), "th_b2c": (2, 128, 1), "tt_b1c": (2, 128, 1),
    "tt_b2c": (2, 128, 1), "ah_bc": (2, 128, 1), "at_bc": (2, 128, 1),
    "eu_b1c": (2, 128, 1), "g_c": (2, 128, 1),
    "kbwk": (8, 1), "b2we128": (128, 1),
    "vbias_rep": (128, D), "obias_rep": (16, D),
    "tb_col": (NSLOT, 1), "alpha_col": (NSLOT, 1), "eps_col": (NSLOT, 1),
    "ones_col": (128, 1), "ones_scl": (128, 1), "ones_row": (1, 128),
    "one1": (1, 1), "sel16": (16, 128), "sel8": (8, 128), "ident": (128, 128),
    "Einv": (NSLOT, S), "Eneg": (NSLOT, S),
    "flag8": (8, 1), "flaginv8": (8, 1), "flag1": (1, 1), "flaginv1": (1, 1),
}


# ---------------------------------------------------------------------------
# numpy mirror of the device program (for fast algebra validation)
# ---------------------------------------------------------------------------
def _mirror_core(m):
    mm = lambda w, x: np.concatenate(
        [w[0].T @ x[:128] + w[1].T @ x[128:]], 0) if False else (w[0].T @ x[:128] + w[1].T @ x[128:])
    relu = lambda x: np.maximum(x, 0.0)
    dTl, dT, nT, nTl = m["desc_T_loc"], m["desc_T"], m["nv_T"], m["nv_T_loc"]
    bias = lambda c: np.concatenate([c[0], c[1]], 0)  # [256,1]
    h1 = relu(mm(m["th_w1"], dTl) + bias(m["th_b1c"]))
    dh = mm(m["th_w2"], h1) + bias(m["th_b2c"])            # [256,65]
    h2 = relu(mm(m["tt_w1"], dT) + bias(m["tt_b1c"]))
    dt = mm(m["tt_w2"], h2) + bias(m["tt_b2c"])            # [256,128]
    vh = mm(m["ah_w"], nTl) + bias(m["ah_bc"])             # [256,65]
    vt = mm(m["at_w"], nT[:, 1:]) + bias(m["at_bc"])       # [256,128]
    QT = mm(m["w1b"], dT) + bias(m["eu_b1c"])              # [256,128]
    PTl = mm(m["w1a"], dTl)                                # [256,65]
    PTf = mm(m["w1a"], dT)                                 # [256,128]
    U = dh.T @ dt                                          # [65,128]
    invnh = 1.0 / np.sqrt((dh * dh).sum(0))                # [65]
    invnt = 1.0 / np.sqrt((dt * dt).sum(0))
    A = 1.0 / (1.0 + np.exp(-(U * invnh[:, None] * invnt[None, :] + m["tb_col"][0, 0])))
    W = vh.T @ vt
    G = A * W * m["Einv"] + m["Eneg"]
    rmax = G.max(1, keepdims=True)
    eG = np.exp(G - rmax)
    adj0 = eG / eG.sum(1, keepdims=True)
    thr = rmax * m["alpha_col"][0, 0]
    mask = (G >= thr).astype(F32)
    adj1 = adj0 * mask
    rs = adj1.sum(1, keepdims=True)
    adjL = adj1 / (rs + (rs == 0.0))
    # P/Q stats
    mQ = QT.mean(0); Qc = QT - mQ
    mPl = PTl.mean(0); Pcl = PTl - mPl
    mPf = PTf.mean(0); Pcf = PTf - mPf
    varQ = (Qc * Qc).mean(0); varPl = (Pcl * Pcl).mean(0); varPf = (Pcf * Pcf).mean(0)
    C = Pcl.T @ Qc                                         # [65,128]
    s2 = C * (2.0 / 256.0) + varPl[:, None] + varQ[None, :]
    invsig = 1.0 / np.sqrt(s2 + 1e-5)                      # [65,128]
    diag = (Pcf * Qc).sum(0)
    invsig_cls = 1.0 / np.sqrt(diag * (2.0 / 256.0) + varPf + varQ + 1e-5)  # [128]
    gc = bias(m["g_c"])                                    # [256,1]
    W2E = np.concatenate([m["w2e"][0], m["w2e"][1]], 0)    # [256,8]
    # edge scores: es[slot, h, j'] = relu(g*(Qc_j + Pc_slot)) . W2E[:,h]
    ES = np.zeros((NSLOT, 8, 128), F32)
    for s in range(NSLOT):
        rg = relu(gc * (Qc + Pcl[:, s:s + 1]))             # [256,128]
        ES[s] = (rg.T @ W2E).T
    rgc = relu(gc * (Qc + Pcf))
    escls = (rgc.T @ W2E).T                                # [8,128]
    ktT = (np.concatenate([m["wk8"][0], m["wk8"][1]], 0).T @ nT) + m["kbwk"]  # [8,129]
    v = nT.T @ np.concatenate([m["v_w"][0], m["v_w"][1]], 0) + m["vbias_rep"][0]  # [129,256]
    f, fi = m["flag1"][0, 0], m["flaginv1"][0, 0]
    # assemble per-slot scores
    attn = np.zeros((NSLOT, 8, NS), F32)
    outr = np.zeros((NSLOT, D), F32)
    b2we = m["b2we128"][:8, 0]
    for s in range(NSLOT):
        if s == 0:
            adjrow = np.concatenate([[0.0], f * np.ones(128, F32) + fi * adjL[0]]).astype(F32)
            sigrow = f * invsig_cls + fi * invsig[0]
            es = f * escls + fi * ES[0]
        else:
            adjrow = np.concatenate([[0.0], adjL[s]]).astype(F32)
            sigrow = invsig[s]
            es = ES[s]
        sc = np.zeros((8, NS), F32)
        sc[:, 1:] = (es * sigrow[None, :] + b2we[:, None]) * adjrow[None, 1:]
        sc += ktT
        sc += np.where(adjrow[None, :] == 0.0, F32(NEG), F32(0.0))
        mx = sc.max(1, keepdims=True)
        e = np.exp(sc - mx)
        at = e / e.sum(1, keepdims=True)
        attn[s] = at
        ctx = np.zeros((8, HD), F32)
        for h in range(H):
            ctx[h] = at[h] @ v[:, h * HD:(h + 1) * HD]
        outr[s] = ctx.reshape(D) @ np.concatenate([m["out_w"][0], m["out_w"][1]], 0) + m["obias_rep"][0]
    return outr, attn


def _assemble(results):
    out = np.empty((B, NS, D), F32)
    attn = np.empty((B, H, NS, NS), F32)
    for core in range(8):
        b, half = core // 2, core % 2
        o65 = results[core]["out_rows"]            # [65,256]
        a65 = results[core]["attn_rh"]             # [65,8,129]
        a65 = np.transpose(a65, (1, 0, 2))         # [8,65,129]
        if half == 0:
            out[b, 0:65] = o65
            attn[b, :, 0:65, :] = a65
        else:
            out[b, 65:129] = o65[1:]
            attn[b, :, 65:129, :] = a65[:, 1:]
    return out, attn


def kernel_numpy(**inputs):
    maps = _prep_core_inputs(inputs)
    results = []
    for m in maps:
        o, a = _mirror_core(m)
        results.append({"out_rows": o, "attn_rh": a})
    return _assemble(results)


# ---------------------------------------------------------------------------
# device program
# ---------------------------------------------------------------------------
_CACHE = {}


def _build_nc():
    import concourse.bass as bass
    import concourse.bacc as bacc
    import concourse.tile as tile
    import concourse.mybir as mybir
    from contextlib import ExitStack

    dt = mybir.dt.float32
    AF = mybir.ActivationFunctionType
    ALU = mybir.AluOpType
    AX = mybir.AxisListType

    nc = bacc.Bacc(None, target_bir_lowering=False, debug=False)
    par = {k: nc.declare_dram_parameter(k, list(v), dt, isOutput=False)
           for k, v in INPUT_SPECS.items()}
    out_rows = nc.declare_dram_parameter("out_rows", [NSLOT, D], dt, isOutput=True)
    attn_rh = nc.declare_dram_parameter("attn_rh", [NSLOT, 8, NS], dt, isOutput=True)

    with ExitStack() as ctx:
        tc = ctx.enter_context(tile.TileContext(nc))
        cpool = ctx.enter_context(tc.tile_pool(name="consts", bufs=1))
        apool = ctx.enter_context(tc.tile_pool(name="acts", bufs=1))
        lpool = ctx.enter_context(tc.tile_pool(name="loop", bufs=2))
        psA = ctx.enter_context(tc.tile_pool(name="psA", bufs=3, space="PSUM"))
        psE = ctx.enter_context(tc.tile_pool(name="psE", bufs=2, space="PSUM"))

        SB = {}  # name -> tile or [tile,tile] for [2,128,n]

        def load(name):
            shp = INPUT_SPECS[name]
            if len(shp) == 3:
                ts = []
                for k in range(2):
                    t = cpool.tile([shp[1], shp[2]], dt, tag=f"{name}{k}")
                    nc.sync.dma_start(out=t[:, :], in_=par[name][k, :, :])
                    ts.append(t)
                SB[name] = ts
            else:
                t = cpool.tile(list(shp), dt, tag=name)
                nc.sync.dma_start(out=t[:, :], in_=par[name][:, :])
                SB[name] = t

        _SPLIT = {"desc_T", "desc_T_loc", "nv_T", "nv_T_loc"}
        for k in INPUT_SPECS:
            if k not in _SPLIT:
                load(k)

        def mmps(tag, lhsTs, rhss, mdim, ndim):
            """accumulating matmul over len(lhsTs) k-tiles -> psum tile [mdim, ndim]"""
            ps = psA.tile([mdim, ndim], dt, tag="mm")
            nk = len(lhsTs)
            for ki in range(nk):
                nc.tensor.matmul(ps[:, :], lhsTs[ki], rhss[ki],
                                 start=(ki == 0), stop=(ki == nk - 1))
            return ps

        def proj(name, wname, in_tiles, n, bias_name=None, func=AF.Identity,
                 in_slice=None):
            """out[2][128,n] = func(W^T x + b), transposed activations"""
            w = SB[wname]
            outs = []
            for mi in range(2):
                lhsTs = [w[ki][:, mi * 128:(mi + 1) * 128] for ki in range(2)]
                rhss = []
                for ki in range(2):
                    src = in_tiles[ki]
                    rhss.append(src[:, in_slice[0]:in_slice[1]] if in_slice else src[:, :n])
                ps = mmps(f"{name}{mi}", lhsTs, rhss, 128, n)
                o = apool.tile([128, n], dt, tag=f"{name}{mi}")
                b = SB[bias_name][mi][:, 0:1] if bias_name else 0.0
                nc.scalar.activation(out=o[:, :], in_=ps[:, :], func=func, bias=b)
                outs.append(o)
            return outs

        # inputs stored as [256, n] need splitting into two [128, n] tiles:
        def split2(name):
            shp = INPUT_SPECS[name]
            ts = []
            for k in range(2):
                t = cpool.tile([128, shp[1]], dt, tag=f"{name}_s{k}")
                nc.sync.dma_start(out=t[:, :], in_=par[name][k * 128:(k + 1) * 128, :])
                ts.append(t)
            return ts
        descT = split2("desc_T")
        descTl = split2("desc_T_loc")
        nvT = split2("nv_T")
        nvTl = split2("nv_T_loc")

        h1 = proj("h1", "th_w1", descTl, NSLOT, "th_b1c", AF.Relu)
        dh = proj("dh", "th_w2", h1, NSLOT, "th_b2c")
        h2 = proj("h2", "tt_w1", descT, 128, "tt_b1c", AF.Relu)
        dtt = proj("dt", "tt_w2", h2, 128, "tt_b2c")
        vh = proj("vh", "ah_w", nvTl, NSLOT, "ah_bc")
        vt = proj("vt", "at_w", nvT, 128, "at_bc", in_slice=(1, 129))
        QT = proj("QT", "w1b", descT, 128, "eu_b1c")
        PTl = proj("PTl", "w1a", descTl, NSLOT)
        PTf = proj("PTf", "w1a", descT, 128)

        ones_col = SB["ones_col"]
        ones_scl = SB["ones_scl"]
        ones_row = SB["ones_row"]

        def colsum_row(tag, tiles, n, scaled):
            o_lhs = ones_scl if scaled else ones_col
            ps = mmps(f"cs_{tag}", [o_lhs[:, 0:1], o_lhs[:, 0:1]],
                      [tiles[0][:, :n], tiles[1][:, :n]], 1, n)
            r = apool.tile([1, n], dt, tag=f"row_{tag}")
            nc.vector.tensor_copy(out=r[:, :], in_=ps[:, :])
            return r

        def square(tag, tiles, n):
            outs = []
            for ki in range(2):
                o = apool.tile([128, n], dt, tag=f"sq_{tag}{ki}")
                nc.vector.tensor_mul(o[:, :], tiles[ki][:, :n], tiles[ki][:, :n])
                outs.append(o)
            return outs

        def center(tag, tiles, n):
            """subtract per-column mean (over 256 partitions)."""
            mrow = colsum_row(tag, tiles, n, scaled=True)
            mrep = mmps(f"rep_{tag}", [ones_row[0:1, :]], [mrow[0:1, :]], 128, n)
            outs = []
            for ki in range(2):
                o = apool.tile([128, n], dt, tag=f"c_{tag}{ki}")
                nc.vector.tensor_sub(o[:, :], tiles[ki][:, :n], mrep[:, :])
                outs.append(o)
            return outs

        Qc = center("Q", QT, 128)
        Pcl = center("Pl", PTl, NSLOT)
        Pcf = center("Pf", PTf, 128)
        varQ = colsum_row("vQ", square("Q", Qc, 128), 128, scaled=True)
        varPl = colsum_row("vPl", square("Pl", Pcl, NSLOT), NSLOT, scaled=True)
        varPf = colsum_row("vPf", square("Pf", Pcf, 128), 128, scaled=True)

        # Pg = Pc * g  (per-partition scale) -> used as relu bias columns
        Pg = []
        for ki in range(2):
            o = apool.tile([128, NSLOT], dt, tag=f"Pg{ki}")
            nc.vector.tensor_scalar_mul(o[:, :], Pcl[ki][:, :], SB["g_c"][ki][:, 0:1])
            Pg.append(o)

        # invsig rows [65,128]
        C = mmps("C", [Pcl[0][:, :], Pcl[1][:, :]], [Qc[0][:, :], Qc[1][:, :]],
                 NSLOT, 128)
        varPl_colps = mmps("vPlc", [varPl[0:1, :]], [SB["one1"][0:1, 0:1]], NSLOT, 1)
        varPl_col = apool.tile([NSLOT, 1], dt, tag="varPl_col")
        nc.vector.tensor_copy(out=varPl_col[:, :], in_=varPl_colps[:, :])
        varQrep = mmps("vQrep", [ones_row[0:1, 0:NSLOT]], [varQ[0:1, :]], NSLOT, 128)
        s2 = apool.tile([NSLOT, 128], dt, tag="s2")
        nc.vector.tensor_scalar(out=s2[:, :], in0=C[:, :], scalar1=2.0 / 256.0,
                                scalar2=varPl_col[:, 0:1], op0=ALU.mult, op1=ALU.add)
        nc.vector.tensor_add(s2[:, :], s2[:, :], varQrep[:, :])
        sig = apool.tile([NSLOT, 128], dt, tag="sig")
        nc.scalar.activation(out=sig[:, :], in_=s2[:, :], func=AF.Sqrt,
                             bias=SB["eps_col"][:, 0:1])
        invsig = apool.tile([NSLOT, 128], dt, tag="invsig")
        nc.vector.reciprocal(out=invsig[:, :], in_=sig[:, :])

        # CLS invsig row [1,128]
        Dg = []
        for ki in range(2):
            o = apool.tile([128, 128], dt, tag=f"Dg{ki}")
            nc.vector.tensor_mul(o[:, :], Pcf[ki][:, :], Qc[ki][:, :])
            Dg.append(o)
        diag = colsum_row("diag", Dg, 128, scaled=True)  # mean(Pc*Qc) = (1/256)*dot
        s2c = apool.tile([1, 128], dt, tag="s2c")
        nc.vector.tensor_scalar_mul(s2c[:, :], diag[0:1, :], 2.0)
        nc.vector.tensor_add(s2c[:, :], s2c[:, :], varPf[0:1, :])
        nc.vector.tensor_add(s2c[:, :], s2c[:, :], varQ[0:1, :])
        sigc = apool.tile([1, 128], dt, tag="sigc")
        nc.scalar.activation(out=sigc[:, :], in_=s2c[:, :], func=AF.Sqrt,
                             bias=SB["eps_col"][0:1, 0:1])
        invsig_cls = apool.tile([1, 128], dt, tag="invsig_cls")
        nc.vector.reciprocal(out=invsig_cls[:, :], in_=sigc[:, :])

        # --- A / G / adj ---
        nh2 = colsum_row("nh", square("dh", dh, NSLOT), NSLOT, scaled=False)
        nt2 = colsum_row("nt", square("dt", dtt, 128), 128, scaled=False)
        nh = apool.tile([1, NSLOT], dt, tag="nh")
        nc.scalar.activation(out=nh[:, :], in_=nh2[:, :], func=AF.Sqrt, bias=0.0)
        invnh = apool.tile([1, NSLOT], dt, tag="invnh")
        nc.vector.reciprocal(out=invnh[:, :], in_=nh[:, :])
        nt = apool.tile([1, 128], dt, tag="nt")
        nc.scalar.activation(out=nt[:, :], in_=nt2[:, :], func=AF.Sqrt, bias=0.0)
        invnt = apool.tile([1, 128], dt, tag="invnt")
        nc.vector.reciprocal(out=invnt[:, :], in_=nt[:, :])
        Rhps = mmps("Rh", [invnh[0:1, :]], [ones_row[0:1, :]], NSLOT, 128)
        Rhsb = apool.tile([NSLOT, 128], dt, tag="Rhsb")
        nc.vector.tensor_copy(out=Rhsb[:, :], in_=Rhps[:, :])
        Rtps = mmps("Rt", [ones_row[0:1, 0:NSLOT]], [invnt[0:1, :]], NSLOT, 128)
        Rtsb = apool.tile([NSLOT, 128], dt, tag="Rtsb")
        nc.vector.tensor_copy(out=Rtsb[:, :], in_=Rtps[:, :])
        U = mmps("U", [dh[0][:, :], dh[1][:, :]], [dtt[0][:, :], dtt[1][:, :]],
                 NSLOT, 128)
        Apre = apool.tile([NSLOT, 128], dt, tag="Apre")
        nc.vector.tensor_mul(Apre[:, :], U[:, :], Rhsb[:, :])
        nc.vector.tensor_mul(Apre[:, :], Apre[:, :], Rtsb[:, :])
        Amat = apool.tile([NSLOT, 128], dt, tag="Amat")
        nc.scalar.activation(out=Amat[:, :], in_=Apre[:, :], func=AF.Sigmoid,
                             bias=SB["tb_col"][:, 0:1])
        Wm = mmps("Wm", [vh[0][:, :], vh[1][:, :]], [vt[0][:, :], vt[1][:, :]],
                  NSLOT, 128)
        G = apool.tile([NSLOT, 128], dt, tag="G")
        nc.vector.tensor_mul(G[:, :], Amat[:, :], Wm[:, :])
        nc.vector.tensor_mul(G[:, :], G[:, :], SB["Einv"][:, :])
        nc.vector.tensor_add(G[:, :], G[:, :], SB["Eneg"][:, :])
        rmax = apool.tile([NSLOT, 1], dt, tag="rmax")
        nc.vector.tensor_reduce(out=rmax[:, :], in_=G[:, :], axis=AX.X, op=ALU.max)
        ngmax = apool.tile([NSLOT, 1], dt, tag="ngmax")
        nc.vector.tensor_scalar_mul(ngmax[:, :], rmax[:, :], -1.0)
        eG = apool.tile([NSLOT, 128], dt, tag="eG")
        nc.scalar.activation(out=eG[:, :], in_=G[:, :], func=AF.Exp,
                             bias=ngmax[:, 0:1])
        rsum = apool.tile([NSLOT, 1], dt, tag="rsum")
        nc.vector.tensor_reduce(out=rsum[:, :], in_=eG[:, :], axis=AX.X, op=ALU.add)
        rinv = apool.tile([NSLOT, 1], dt, tag="rinv")
        nc.vector.reciprocal(out=rinv[:, :], in_=rsum[:, :])
        adj0 = apool.tile([NSLOT, 128], dt, tag="adj0")
        nc.vector.tensor_scalar_mul(adj0[:, :], eG[:, :], rinv[:, 0:1])
        thr = apool.tile([NSLOT, 1], dt, tag="thr")
        nc.vector.tensor_mul(thr[:, :], rmax[:, :], SB["alpha_col"][:, :])
        maskge = apool.tile([NSLOT, 128], dt, tag="maskge")
        nc.vector.tensor_scalar(out=maskge[:, :], in0=G[:, :], scalar1=thr[:, 0:1],
                                scalar2=None, op0=ALU.is_ge)
        adj1 = apool.tile([NSLOT, 128], dt, tag="adj1")
        nc.vector.tensor_mul(adj1[:, :], adj0[:, :], maskge[:, :])
        rs = apool.tile([NSLOT, 1], dt, tag="rs")
        nc.vector.tensor_reduce(out=rs[:, :], in_=adj1[:, :], axis=AX.X, op=ALU.add)
        rs0 = apool.tile([NSLOT, 1], dt, tag="rs0")
        nc.vector.tensor_scalar(out=rs0[:, :], in0=rs[:, :], scalar1=0.0,
                                scalar2=None, op0=ALU.is_equal)
        den = apool.tile([NSLOT, 1], dt, tag="den")
        nc.vector.tensor_add(den[:, :], rs[:, :], rs0[:, :])
        dinv = apool.tile([NSLOT, 1], dt, tag="dinv")
        nc.vector.reciprocal(out=dinv[:, :], in_=den[:, :])
        adjL = apool.tile([NSLOT, 128], dt, tag="adjL")
        nc.vector.tensor_scalar_mul(adjL[:, :], adj1[:, :], dinv[:, 0:1])

        # --- kterm / v ---
        ktps = mmps("kt", [SB["wk8"][0][:, :], SB["wk8"][1][:, :]],
                    [nvT[0][:, :], nvT[1][:, :]], 8, NS)
        ktT = apool.tile([8, NS], dt, tag="ktT")
        nc.vector.tensor_scalar_add(ktT[:, :], ktps[:, :], SB["kbwk"][:, 0:1])
        ktrep_ps = mmps("ktrep", [SB["sel8"][:, :]], [ktT[:, :]], 128, NS)
        ktrep = apool.tile([128, NS], dt, tag="ktrep")
        nc.vector.tensor_copy(out=ktrep[:, :], in_=ktrep_ps[:, :])

        vaps = mmps("va", [nvT[0][:, 0:128], nvT[1][:, 0:128]],
                    [SB["v_w"][0][:, :], SB["v_w"][1][:, :]], 128, D)
        va = apool.tile([128, D], dt, tag="va")
        nc.vector.tensor_add(va[:, :], vaps[:, :], SB["vbias_rep"][:, :])
        vbps = mmps("vb", [nvT[0][:, 128:129], nvT[1][:, 128:129]],
                    [SB["v_w"][0][:, :], SB["v_w"][1][:, :]], 1, D)
        vb = apool.tile([1, D], dt, tag="vb")
        nc.vector.tensor_add(vb[:, :], vbps[:, :], SB["vbias_rep"][0:1, :])

        # --- CLS edge scores ---
        rgc = []
        for ki in range(2):
            t = apool.tile([128, 128], dt, tag=f"tcls{ki}")
            nc.vector.tensor_add(t[:, :], Pcf[ki][:, :], Qc[ki][:, :])
            r = apool.tile([128, 128], dt, tag=f"rgc{ki}")
            nc.scalar.activation(out=r[:, :], in_=t[:, :], func=AF.Relu,
                                 scale=SB["g_c"][ki][:, 0:1])
            rgc.append(r)
        escls_ps = mmps("escls", [SB["w2e"][0][:, :], SB["w2e"][1][:, :]],
                        [rgc[0][:, :], rgc[1][:, :]], 8, 128)
        escls = apool.tile([8, 128], dt, tag="escls")
        nc.vector.tensor_copy(out=escls[:, :], in_=escls_ps[:, :])

        # --- per-block attention ---
        blocks = [(0, 16), (16, 16), (32, 16), (48, 16), (64, 1)]
        for (r0, cnt) in blocks:
            p8 = 8 * cnt
            ES = lpool.tile([128, 128], dt, tag="ES")
            esb = lpool.tile([8, 2048], dt, tag="esb")
            ngrp = (cnt + 3) // 4
            for gi in range(ngrp):
                gs = list(range(r0 + gi * 4, min(r0 + gi * 4 + 4, r0 + cnt)))
                gw = len(gs) * 128
                rg0 = lpool.tile([128, 512], dt, tag="rg0")
                rg1 = lpool.tile([128, 512], dt, tag="rg1")
                for idx, s in enumerate(gs):
                    for ki, rg in ((0, rg0), (1, rg1)):
                        nc.scalar.activation(
                            out=rg[:, idx * 128:(idx + 1) * 128],
                            in_=Qc[ki][:, :], func=AF.Relu,
                            scale=SB["g_c"][ki][:, 0:1],
                            bias=Pg[ki][:, s:s + 1])
                es_ps = psE.tile([8, 512], dt, tag="es")
                nc.tensor.matmul(es_ps[:, 0:gw], SB["w2e"][0][:, :], rg0[:, 0:gw],
                                 start=True, stop=False)
                nc.tensor.matmul(es_ps[:, 0:gw], SB["w2e"][1][:, :], rg1[:, 0:gw],
                                 start=False, stop=True)
                nc.vector.tensor_copy(out=esb[:, gi * 512:gi * 512 + gw],
                                      in_=es_ps[:, 0:gw])
            if r0 == 0:
                # blend slot0 with CLS path
                tmp8 = lpool.tile([8, 128], dt, tag="tmp8")
                nc.vector.tensor_scalar_mul(tmp8[:, :], esb[:, 0:128],
                                            SB["flaginv8"][:, 0:1])
                nc.vector.tensor_scalar_mul(esb[:, 0:128], escls[:, :],
                                            SB["flag8"][:, 0:1])
                nc.vector.tensor_add(esb[:, 0:128], esb[:, 0:128], tmp8[:, :])
            for h in range(H):
                nc.sync.dma_start(
                    out=ES[h:8 * cnt:8, :],
                    in_=esb[h:h + 1, 0:cnt * 128].rearrange(
                        "p (r j) -> p r j", j=128))

            adj16 = lpool.tile([16, NS], dt, tag="adj16")
            nc.vector.memset(adj16[0:cnt, 0:1], 0.0)
            nc.sync.dma_start(out=adj16[0:cnt, 1:NS], in_=adjL[r0:r0 + cnt, :])
            sig16 = lpool.tile([16, 128], dt, tag="sig16")
            nc.sync.dma_start(out=sig16[0:cnt, :], in_=invsig[r0:r0 + cnt, :])
            if r0 == 0:
                nc.vector.tensor_scalar(out=adj16[0:1, 1:NS], in0=adjL[0:1, :],
                                        scalar1=SB["flaginv1"][0:1, 0:1],
                                        scalar2=SB["flag1"][0:1, 0:1],
                                        op0=ALU.mult, op1=ALU.add)
                tmp1 = lpool.tile([1, 128], dt, tag="tmp1")
                nc.vector.tensor_scalar_mul(tmp1[:, :], invsig_cls[:, :],
                                            SB["flag1"][0:1, 0:1])
                nc.vector.tensor_scalar_mul(sig16[0:1, :], invsig[0:1, :],
                                            SB["flaginv1"][0:1, 0:1])
                nc.vector.tensor_add(sig16[0:1, :], sig16[0:1, :], tmp1[:, :])

            ADJ = psA.tile([128, NS], dt, tag="mm")
            nc.tensor.matmul(ADJ[:, :], SB["sel16"][0:cnt, :], adj16[0:cnt, :],
                             start=True, stop=True)
            SIG = psA.tile([128, 128], dt, tag="mm")
            nc.tensor.matmul(SIG[:, :], SB["sel16"][0:cnt, :], sig16[0:cnt, :],
                             start=True, stop=True)

            sc = lpool.tile([128, NS], dt, tag="scores")
            nc.vector.tensor_mul(sc[0:p8, 1:NS], ES[0:p8, :], SIG[0:p8, :])
            nc.vector.tensor_scalar_add(sc[0:p8, 1:NS], sc[0:p8, 1:NS],
                                        SB["b2we128"][0:p8, 0:1])
            nc.vector.tensor_mul(sc[0:p8, 1:NS], sc[0:p8, 1:NS], ADJ[0:p8, 1:NS])
            nc.vector.tensor_add(sc[0:p8, 1:NS], sc[0:p8, 1:NS], ktrep[0:p8, 1:NS])
            nc.vector.tensor_copy(out=sc[0:p8, 0:1], in_=ktrep[0:p8, 0:1])
            negt = lpool.tile([128, NS], dt, tag="negt")
            nc.vector.tensor_scalar(out=negt[0:p8, :], in0=ADJ[0:p8, :],
                                    scalar1=0.0, scalar2=float(NEG),
                                    op0=ALU.is_equal, op1=ALU.mult)
            nc.vector.tensor_add(sc[0:p8, :], sc[0:p8, :], negt[0:p8, :])

            smax = lpool.tile([128, 1], dt, tag="smax")
            nc.vector.tensor_reduce(out=smax[0:p8, :], in_=sc[0:p8, :], axis=AX.X,
                                    op=ALU.max)
            nsmax = lpool.tile([128, 1], dt, tag="nsmax")
            nc.vector.tensor_scalar_mul(nsmax[0:p8, :], smax[0:p8, :], -1.0)
            at = lpool.tile([128, NS], dt, tag="at")
            nc.scalar.activation(out=at[0:p8, :], in_=sc[0:p8, :], func=AF.Exp,
                                 bias=nsmax[0:p8, 0:1])
            ssum = lpool.tile([128, 1], dt, tag="ssum")
            nc.vector.tensor_reduce(out=ssum[0:p8, :], in_=at[0:p8, :], axis=AX.X,
                                    op=ALU.add)
            sinv = lpool.tile([128, 1], dt, tag="sinv")
            nc.vector.reciprocal(out=sinv[0:p8, :], in_=ssum[0:p8, :])
            nc.vector.tensor_scalar_mul(at[0:p8, :], at[0:p8, :], sinv[0:p8, 0:1])

            nc.sync.dma_start(out=attn_rh[r0:r0 + cnt, :, :], in_=at[0:p8, :])

            # transpose attn -> [j, (r h)]
            Ta = psA.tile([128, 128], dt, tag="mm")
            nc.tensor.transpose(Ta[:, 0:p8], at[0:p8, 0:128], SB["ident"][0:p8, 0:p8])
            Tb = psA.tile([1, 128], dt, tag="mm")
            nc.tensor.transpose(Tb[0:1, 0:p8], at[0:p8, 128:129],
                                SB["ident"][0:p8, 0:p8])
            Tasb = lpool.tile([128, 128], dt, tag="Tasb")
            nc.vector.tensor_copy(out=Tasb[:, 0:p8], in_=Ta[:, 0:p8])
            Tbsb = lpool.tile([1, 128], dt, tag="Tbsb")
            nc.vector.tensor_copy(out=Tbsb[:, 0:p8], in_=Tb[:, 0:p8])

            ctxT0 = lpool.tile([128, 16], dt, tag="ctxT0")
            ctxT1 = lpool.tile([128, 16], dt, tag="ctxT1")
            Tar = Tasb[:, 0:p8].rearrange("j (r h) -> j h r", h=8)
            Tbr = Tbsb[:, 0:p8].rearrange("j (r h) -> j h r", h=8)
            for h in range(H):
                cps = psA.tile([HD, 16], dt, tag="mm")
                nc.tensor.matmul(cps[:, 0:cnt], va[:, h * HD:(h + 1) * HD],
                                 Tar[:, h, :], start=True, stop=False)
                nc.tensor.matmul(cps[:, 0:cnt], vb[0:1, h * HD:(h + 1) * HD],
                                 Tbr[:, h, :], start=False, stop=True)
                dst = ctxT0 if h < 4 else ctxT1
                nc.vector.tensor_copy(out=dst[HD * (h % 4):HD * (h % 4 + 1), 0:cnt],
                                      in_=cps[:, 0:cnt])
            ops = psA.tile([16, D], dt, tag="mm")
            nc.tensor.matmul(ops[0:cnt, :], ctxT0[:, 0:cnt], SB["out_w"][0][:, :],
                             start=True, stop=False)
            nc.tensor.matmul(ops[0:cnt, :], ctxT1[:, 0:cnt], SB["out_w"][1][:, :],
                             start=False, stop=True)
            orow = lpool.tile([16, D], dt, tag="orow")
            nc.vector.tensor_add(orow[0:cnt, :], ops[0:cnt, :],
                                 SB["obias_rep"][0:cnt, :])
            nc.sync.dma_start(out=out_rows[r0:r0 + cnt, :], in_=orow[0:cnt, :])

    nc.finalize()
    return nc


def kernel(**inputs):
    import os
    from concourse.bass_utils import run_bass_kernel_spmd
    maps = _prep_core_inputs(inputs)
    if "nc" not in _CACHE:
        _CACHE["nc"] = _build_nc()
    trace = bool(int(os.environ.get("KTRACE", "0")))
    res = run_bass_kernel_spmd(_CACHE["nc"], maps, core_ids=list(range(8)),
                               trace=trace)
    _CACHE["exec_time_ns"] = res.exec_time_ns
    return _assemble(res.results)


if __name__ == "__main__":
    import reference
    inputs = {k: np.asarray(v) for k, v in reference.setup_inputs().items()}
    exp_out, exp_attn = [np.asarray(x) for x in reference.reference(**inputs)]
    o, a = kernel_numpy(**inputs)
    eo = np.linalg.norm(o - exp_out) / np.linalg.norm(exp_out)
    ea = np.linalg.norm(a - exp_attn) / np.linalg.norm(exp_attn)
    print(f"numpy mirror rel err out={eo:.3e} attn={ea:.3e}")
